# revision 15
# baseline (speedup 1.0000x reference)
"""Multi-head attention (B=2, S=2048, D=1024, H=16, causal) on 8 trn2 cores.

Sharding: per-batch head-parallel. Core c handles batch c//4 and head group
c%4 (4 heads). All matmul datapaths run in bf16 (fp32 PSUM accumulate).
Attention runs fully on-core with causal block skipping at two granularities:
scores are computed in [128k x 512q] tiles (additive -1e9 masks only on the
128x128 diagonal sub-blocks), and the attn@V contraction is flipped to
out[q, dk] form so each 128x128 q/k sub-block costs 65 PE rows instead of
128 and fully-masked sub-blocks are skipped. Softmax denominators ride along
as a ones-column of V (column DK); per-q reciprocals are applied on the DVE
and the context is transposed back to [dk, q] with PE transposes. The context
is exchanged with two 8-core AllToAlls (one per head-pair, issued as each
pair completes so the collectives overlap the remaining attention compute).
Each core finishes the output projection for a disjoint block of 512 rows
(batch c//4, query quarter c%4); batch selection after the exchange uses a
per-core indirect gather so the SPMD program is identical on every core.
"""

import sys

sys.path.insert(0, "/opt/trn_rl_repo")

import math

import numpy as np
import ml_dtypes

import concourse.bass as bass
import concourse.bacc as bacc
import concourse.mybir as mybir
from concourse.tile import TileContext, add_dep_helper
from concourse.bass_utils import run_bass_kernel_spmd

F32 = mybir.dt.float32
BF16 = mybir.dt.bfloat16
AF = mybir.ActivationFunctionType
NPBF16 = ml_dtypes.bfloat16

B, S, D, H, DK = 2, 2048, 1024, 16, 64
NCORES = 8
HPC = 4  # heads per core
JPC = HPC * DK  # 256 j-dims per core
QT = 512  # q tile (free dim of scores)
KT = 128  # k tile (partition dim of scores)
QS = 128  # q sub-block (ctx granularity)
NQC = S // QT  # 4
NKC = S // KT  # 16
NQI = S // QS  # 16
NDC = D // 128  # 8 contraction chunks for projections
QUAD = 2  # score blocks per psum tile / exp call

_CACHE: dict = {}


def _build_program(mask_info, reps, zero_bias=False):
    """mask_info: (score_blocks, mask_adds, ctx_blocks, n_masks); uniform
    across cores.
      score_blocks[qc] = [kc, ...] score tiles to compute
      mask_adds[qc] = [(kc, qsub, tile_idx), ...] 0/1 st masks (128x128)
      ctx_blocks[qi] = [kc, ...] sub-blocks contributing to ctx of q-chunk qi
    zero_bias: skip the bv/bo bias matmuls (biases known to be all-zero).
    """
    score_blocks, mask_adds, ctx_blocks, n_masks = mask_info
    nc = bacc.Bacc()

    xq = nc.declare_dram_parameter("xq", [D, S], BF16, isOutput=False)
    xk = nc.declare_dram_parameter("xk", [D, S], BF16, isOutput=False)
    xv = nc.declare_dram_parameter("xv", [D, S], BF16, isOutput=False)
    wq = nc.declare_dram_parameter("wq", [D, JPC], BF16, isOutput=False)
    wk = nc.declare_dram_parameter("wk", [D, JPC], BF16, isOutput=False)
    wv = nc.declare_dram_parameter("wv", [D, JPC], BF16, isOutput=False)
    wo = nc.declare_dram_parameter("wo", [D, D], BF16, isOutput=False)
    cxidx = nc.declare_dram_parameter("cxidx", [128, 4], mybir.dt.int32, isOutput=False)
    mka = nc.declare_dram_parameter(
        "maskadd", [max(n_masks, 1), KT, QS], BF16, isOutput=False
    )
    ident = nc.declare_dram_parameter("ident", [128, 128], BF16, isOutput=False)
    onesr = nc.declare_dram_parameter("onesrow", [1, 128], BF16, isOutput=False)
    onesc = nc.declare_dram_parameter("onescol", [128, 64], BF16, isOutput=False)
    bq = nc.declare_dram_parameter("bq", [128, 2], F32, isOutput=False)
    bk = nc.declare_dram_parameter("bk", [128, 2], F32, isOutput=False)
    bv = nc.declare_dram_parameter("bv", [1, JPC], BF16, isOutput=False)
    bo = nc.declare_dram_parameter("bo", [1, D], BF16, isOutput=False)
    out = nc.declare_dram_parameter("o", [S // NQC, D], BF16, isOutput=True)

    # per-head-pair exchange buffers: shard r = [128 (2 heads x 64 dk), 512 q]
    cc_in = [nc.dram_tensor(f"cc_in{p}", [NCORES, 128, QT], BF16) for p in range(2)]
    cc_all = [nc.dram_tensor(f"cc_all{p}", [NCORES, 128, QT], BF16) for p in range(2)]

    with TileContext(nc) as tc:
        with (
            tc.tile_pool(name="persist", bufs=1) as pp,
            tc.tile_pool(name="consts", bufs=1) as cp,
        ):
            # constants
            ident_sb = cp.tile([128, 128], BF16)
            nc.gpsimd.dma_start(out=ident_sb[:], in_=ident[:])
            ones_sb = cp.tile([1, 128], BF16)
            nc.gpsimd.dma_start(out=ones_sb[:], in_=onesr[:])
            onescol_sb = cp.tile([128, 64], BF16)
            nc.gpsimd.dma_start(out=onescol_sb[:], in_=onesc[:])
            mka_sb = cp.tile([128, max(n_masks, 1), QS], BF16)
            nc.gpsimd.dma_start(out=mka_sb[:], in_=mka[:].rearrange("n p q -> p n q"))
            bq_sb = cp.tile([128, 2], F32)
            nc.gpsimd.dma_start(out=bq_sb[:], in_=bq[:])
            bk_sb = cp.tile([128, 2], F32)
            nc.gpsimd.dma_start(out=bk_sb[:], in_=bk[:])
            bv_sb = cp.tile([1, JPC], BF16)
            nc.gpsimd.dma_start(out=bv_sb[:], in_=bv[:])
            bo_sb = cp.tile([1, D], BF16)
            nc.gpsimd.dma_start(out=bo_sb[:], in_=bo[:])
            cxidx_sb = cp.tile([128, 4], mybir.dt.int32)
            nc.gpsimd.dma_start(out=cxidx_sb[:], in_=cxidx[:])

            # persistent activations
            qt_sb = [pp.tile([128, S], BF16, tag=f"qt{j}", name=f"qt{j}") for j in range(2)]
            kt_sb = [pp.tile([128, S], BF16, tag=f"kt{j}", name=f"kt{j}") for j in range(2)]
            vh_sb = pp.tile([128, NKC, HPC * (DK + 1)], BF16, tag="vh")
            # two heads stacked per pair: head 2p in partitions 0-63, 2p+1 in 64-127
            ctxt_pair = [
                pp.tile([128, S], BF16, tag=f"ctxt{p}", name=f"ctxt{p}")
                for p in range(2)
            ]

            for _rep in range(reps):
                cc_insts = []
                with (
                    tc.tile_pool(name="wslice", bufs=1) as wp,
                    tc.tile_pool(name="attn_sb", bufs=1) as asb,
                    tc.tile_pool(name="scB_ps", bufs=2, space="PSUM") as scB,
                    tc.tile_pool(name="ctB_ps", bufs=1, space="PSUM") as ctB,
                    tc.tile_pool(name="tpB_ps", bufs=1, space="PSUM") as tpB,
                ):
                    wq_sb = wp.tile([128, NDC, JPC], BF16, tag="wq")
                    nc.sync.dma_start(
                        out=wq_sb[:], in_=wq[:].rearrange("(a p) j -> p a j", p=128)
                    )
                    wk_sb = wp.tile([128, NDC, JPC], BF16, tag="wk")
                    nc.scalar.dma_start(
                        out=wk_sb[:], in_=wk[:].rearrange("(a p) j -> p a j", p=128)
                    )
                    xk_sb = wp.tile([128, NDC, S], BF16, tag="xk_sb")
                    xk_r = xk[:].rearrange("(a p) s -> p a s", p=128)
                    nc.scalar.dma_start(
                        out=xk_sb[:, :, 0:QT], in_=xk_r[:, :, 0:QT]
                    )
                    xq_sb = wp.tile([128, NDC, S], BF16, tag="xq_sb")
                    for kc in range(NDC):
                        nc.sync.dma_start(
                            out=xq_sb[:, kc, :],
                            in_=xq[kc * 128 : (kc + 1) * 128, :],
                        )
                    for sq in range(1, 4):
                        nc.scalar.dma_start(
                            out=xk_sb[:, :, sq * QT : (sq + 1) * QT],
                            in_=xk_r[:, :, sq * QT : (sq + 1) * QT],
                        )
                    wv_sb = wp.tile([128, NDC, JPC], BF16, tag="wv")
                    nc.gpsimd.dma_start(
                        out=wv_sb[:], in_=wv[:].rearrange("(a p) j -> p a j", p=128)
                    )
                    xvt = wp.tile([128, NDC, S], BF16, tag="xvt")
                    for scg in range(4):
                        nc.gpsimd.dma_start(
                            out=xvt[:, :, scg * 512 : (scg + 1) * 512],
                            in_=xv[:].rearrange("(a p) s -> p a s", p=128)[
                                :, :, scg * 512 : (scg + 1) * 512
                            ],
                        )

                    # ones column for the denominator trick (column DK of each
                    # per-head (DK+1)-stride slot)
                    nc.vector.tensor_copy(
                        vh_sb[:]
                        .rearrange("p s (h e) -> p s h e", e=DK + 1)[
                            :, :, :, DK : DK + 1
                        ],
                        onescol_sb[:]
                        .rearrange("p (s h) -> p s h", s=NKC)
                        .unsqueeze(3),
                    )

                    # ------------- filler chains (share "sc" psum) ---------
                    def emit_qkchain(jg, w_sb, x_sb, b_sb, t_sb, qs):
                        qps = scB.tile([128, 2 * QT], F32, tag="sc", name="qps")
                        for kc in range(NDC):
                            nc.tensor.matmul(
                                qps[:, 0:QT],
                                w_sb[:, kc, jg * 128 : (jg + 1) * 128],
                                x_sb[:, kc, qs * QT : (qs + 1) * QT],
                                start=(kc == 0),
                                stop=(kc == NDC - 1),
                            )
                        nc.vector.tensor_scalar_add(
                            t_sb[:, qs * QT : (qs + 1) * QT],
                            qps[:, 0:QT],
                            b_sb[:, jg : jg + 1],
                        )

                    halves = {}

                    def emit_qkhalf(jg, w_sb, x_sb, b_sb, t_sb, qs, lo):
                        """Half of a jg-1 projection chain (4 kc terms) as a
                        small PE filler; the two halves are merged with one
                        DVE add (requires zero bias)."""
                        qps = scB.tile([128, 2 * QT], F32, tag="sc", name="qps")
                        for kc in range(4 * lo, 4 * lo + 4):
                            nc.tensor.matmul(
                                qps[:, 0:QT],
                                w_sb[:, kc, jg * 128 : (jg + 1) * 128],
                                x_sb[:, kc, qs * QT : (qs + 1) * QT],
                                start=(kc == 4 * lo),
                                stop=(kc == 4 * lo + 3),
                            )
                        key = (id(t_sb), qs)
                        if key in halves:
                            stage = halves.pop(key)
                            nc.vector.tensor_add(
                                t_sb[:, qs * QT : (qs + 1) * QT],
                                stage[:],
                                qps[:, 0:QT],
                            )
                        else:
                            stage = asb.tile(
                                [128, QT], F32, tag="pstage", bufs=2,
                                name="pstage",
                            )
                            nc.vector.tensor_copy(stage[:], qps[:, 0:QT])
                            halves[key] = stage

                    def emit_vchain(sc):
                        vps = scB.tile([128, 2 * QT], F32, tag="sc", name="vps")
                        for kc in range(NDC):
                            nc.tensor.matmul(
                                vps[:, 0:JPC],
                                xvt[:, kc, sc * 128 : (sc + 1) * 128],
                                wv_sb[:, kc, :],
                                start=(kc == 0),
                                stop=(kc == NDC - 1) and zero_bias,
                            )
                        if not zero_bias:
                            nc.tensor.matmul(
                                vps[:, 0:JPC], ones_sb[:], bv_sb[:],
                                start=False, stop=True,
                            )
                        nc.vector.tensor_copy(
                            vh_sb[:, sc, :].rearrange("p (h e) -> p h e", h=HPC)[
                                :, :, 0:DK
                            ],
                            vps[:, 0:JPC].rearrange("p (h e) -> p h e", h=HPC),
                        )

                    # ---------------- attention helpers ----------------
                    sts_map = {}

                    def emit_scores(h, qc, fillers=None):
                        jg, off = h // 2, (h % 2) * DK
                        blocks = score_blocks[qc]
                        madds = mask_adds[qc]
                        sts = []
                        for q0 in range(0, len(blocks), QUAD):
                            if fillers:
                                fillers.popleft()()
                            qd = blocks[q0 : q0 + QUAD]
                            ps = scB.tile(
                                [128, QUAD * QT], F32, tag="sc", name="sc"
                            )
                            st = asb.tile(
                                [128, QUAD, QT], BF16, tag="st", bufs=10,
                                name="st",
                            )
                            sts.append(st)
                            for i, kc in enumerate(qd):
                                nc.tensor.matmul(
                                    ps[:, i * QT : (i + 1) * QT],
                                    kt_sb[jg][
                                        off : off + DK,
                                        kc * KT : (kc + 1) * KT,
                                    ],
                                    qt_sb[jg][
                                        off : off + DK,
                                        qc * QT : (qc + 1) * QT,
                                    ],
                                    start=True,
                                    stop=True,
                                )
                            nc.scalar.activation(
                                st[:, 0 : len(qd), :],
                                ps[:, 0 : len(qd) * QT].rearrange(
                                    "p (n q) -> p n q", q=QT
                                ),
                                AF.Exp,
                                scale=1.0 / math.sqrt(DK),
                            )
                            # zero masked positions of partially-masked
                            # 128x128 sub-blocks (0/1 multiplicative mask)
                            for kc, qsub, mi in madds:
                                if kc in qd:
                                    i = qd.index(kc)
                                    sl = st[:, i, qsub * QS : (qsub + 1) * QS]
                                    nc.vector.tensor_mul(
                                        sl, sl, mka_sb[:, mi, :]
                                    )
                        sts_map[(h, qc)] = sts

                    def emit_ctx(h, qc):
                        ctxt = ctxt_pair[h // 2]
                        coff = (h % 2) * DK
                        hp = h // 2
                        vsl = slice(h * (DK + 1), (h + 1) * (DK + 1))
                        blocks = score_blocks[qc]
                        kpos = {kc: i for i, kc in enumerate(blocks)}
                        sts = sts_map.pop((h, qc))
                        for qsub in range(NQC):
                            qi = qc * 4 + qsub
                            cblocks = ctx_blocks[qi]
                            cps = ctB.tile(
                                [128, DK + 1], F32, tag="ctx", bufs=2,
                                name="ctx",
                            )
                            for ci, kc in enumerate(cblocks):
                                j = kpos[kc]
                                st = sts[j // QUAD]
                                nc.tensor.matmul(
                                    cps[:],
                                    st[:, j % QUAD, qsub * QS : (qsub + 1) * QS],
                                    vh_sb[:, kc, vsl],
                                    start=(ci == 0),
                                    stop=(ci == len(cblocks) - 1),
                                )
                            recip = asb.tile(
                                [128, 1], F32, tag="recip", bufs=4,
                                name="recip",
                            )
                            nc.vector.reciprocal(recip[:], cps[:, DK : DK + 1])
                            ctxq = asb.tile(
                                [128, DK], BF16, tag="ctxq", bufs=4,
                                name="ctxq",
                            )
                            nc.vector.tensor_scalar_mul(
                                ctxq[:], cps[:, 0:DK], recip[:, 0:1]
                            )
                            tps = tpB.tile(
                                [DK, QS], BF16, tag="tps", bufs=2, name="tps"
                            )
                            nc.tensor.transpose(tps[:], ctxq[:], ident_sb[:])
                            nc.vector.tensor_copy(
                                ctxt[coff : coff + DK, qi * QS : (qi + 1) * QS],
                                tps[:],
                            )
                        if h % 2 == 1:
                            # quarter qc of the pair is complete: ship its two
                            # cc_in shards now so the A2A issue isn't gated on
                            # 8 back-to-back staging DMAs at pair end
                            for r in (qc, qc + 4):
                                nc.sync.dma_start(
                                    out=cc_in[hp][r],
                                    in_=ctxt[:, qc * QT : (qc + 1) * QT],
                                )

                    def emit_exchange(hp):
                        cc_insts.append(
                            nc.gpsimd.collective_compute(
                                "AllToAll",
                                mybir.AluOpType.bypass,
                                replica_groups=[list(range(NCORES))],
                                ins=[cc_in[hp][:]],
                                outs=[cc_all[hp][:]],
                            )
                        )

                    # ------- software-pipelined schedule -------------------
                    # scores are Act(exp)-paced; ctx chains run one qc behind
                    # and V/jg1-projection chains fill the remaining PE slots
                    # one per score group.
                    from collections import deque

                    def emit_qproj0_interleaved():
                        """First Q projection, 4 q-chains advanced together at
                        kc granularity so each xq chunk is consumed as it
                        lands, with junk matmuls filling the DMA-paced gaps to
                        keep the PE p-state ramp warm."""
                        qps = [
                            scB.tile([128, 2 * QT], F32, tag="sc", name="qps")
                            for _ in range(2)
                        ]
                        for kc in range(NDC):
                            for qs in range(NQC):
                                nc.tensor.matmul(
                                    qps[qs // 2][:, (qs % 2) * QT : (qs % 2 + 1) * QT],
                                    wq_sb[:, kc, 0:128],
                                    xq_sb[:, kc, qs * QT : (qs + 1) * QT],
                                    start=(kc == 0),
                                    stop=(kc == NDC - 1),
                                )
                            jnk = ctB.tile(
                                [128, DK + 1], F32, tag="ctx", bufs=2, name="jnk"
                            )
                            for _ in range(12):
                                nc.tensor.matmul(
                                    jnk[0:64, :],
                                    ident_sb[0:1, 0:64],
                                    ident_sb[0:1, 0:65],
                                    start=True,
                                    stop=True,
                                )
                        for h2_ in range(2):
                            nc.vector.tensor_scalar_add(
                                qt_sb[0][:, h2_ * 1024 : (h2_ + 1) * 1024],
                                qps[h2_][:],
                                bq_sb[:, 0:1],
                            )

                    f0 = deque(
                        [(lambda sc=sc: emit_vchain(sc)) for sc in range(NKC)]
                    )
                    if zero_bias:
                        f1 = deque(
                            [
                                (lambda qs=qs, lo=lo: emit_qkhalf(
                                    1, wq_sb, xq_sb, bq_sb, qt_sb[1], qs, lo
                                ))
                                for qs in range(NQC)
                                for lo in range(2)
                            ]
                            + [
                                (lambda qs=qs, lo=lo: emit_qkhalf(
                                    1, wk_sb, xk_sb, bk_sb, kt_sb[1], qs, lo
                                ))
                                for qs in range(NQC)
                                for lo in range(2)
                            ]
                        )
                    else:
                        f1 = deque(
                            [
                                (lambda qs=qs: emit_qkchain(
                                    1, wq_sb, xq_sb, bq_sb, qt_sb[1], qs
                                ))
                                for qs in range(NQC)
                            ]
                            + [
                                (lambda qs=qs: emit_qkchain(
                                    1, wk_sb, xk_sb, bk_sb, kt_sb[1], qs
                                ))
                                for qs in range(NQC)
                            ]
                        )
                    emit_qproj0_interleaved()
                    for qs in range(NQC):
                        emit_qkchain(0, wk_sb, xk_sb, bk_sb, kt_sb[0], qs)
                    emit_scores(0, 0, f0)
                    emit_scores(0, 1, f0)
                    emit_ctx(0, 0)
                    emit_scores(0, 2, f0)
                    emit_ctx(0, 1)
                    emit_scores(0, 3, f0)
                    emit_ctx(0, 2)
                    emit_scores(1, 0, f0)
                    emit_ctx(0, 3)
                    emit_scores(1, 1, f0)
                    emit_ctx(1, 0)
                    emit_scores(1, 2, f1)
                    emit_ctx(1, 1)
                    emit_scores(1, 3, f1)
                    emit_ctx(1, 2)
                    while f0:
                        f0.popleft()()
                    emit_ctx(1, 3)
                    emit_exchange(0)
                    emit_scores(2, 0, f1)
                    emit_scores(2, 1, f1)
                    emit_ctx(2, 0)
                    emit_scores(2, 2, f1)
                    emit_ctx(2, 1)
                    emit_scores(2, 3, f1)
                    emit_ctx(2, 2)
                    while f1:
                        f1.popleft()()
                    emit_scores(3, 0)
                    emit_ctx(2, 3)
                    emit_scores(3, 1)
                    emit_ctx(3, 0)
                    emit_scores(3, 2)
                    emit_ctx(3, 1)
                    emit_scores(3, 3)
                    emit_ctx(3, 2)
                    emit_ctx(3, 3)
                    emit_exchange(1)

                # ---------------- output projection (two waves) --------
                with (
                    tc.tile_pool(name="oproj", bufs=1) as op_,
                    tc.tile_pool(name="wo_sbp", bufs=1) as wohp,
                    tc.tile_pool(name="o_stage", bufs=2) as osp,
                    tc.tile_pool(name="o_ps", bufs=1, space="PSUM") as opp,
                ):
                    wo_sb = wohp.tile([128, 8, D], BF16, tag="wo")
                    nc.sync.dma_start(
                        out=wo_sb[:],
                        in_=wo[:].rearrange("(a p) d -> p a d", p=128),
                    )
                    ops = [
                        opp.tile([128, QT], F32, tag=f"ops{t}", name=f"ops{t}")
                        for t in range(8)
                    ]
                    cx = [
                        op_.tile([128, 4, QT], BF16, tag=f"cx{p}", name=f"cx{p}")
                        for p in range(2)
                    ]
                    for hp in range(2):
                        cc_flat = cc_all[hp][:].rearrange("r p q -> (r p) q")
                        for g in range(4):
                            gi = nc.gpsimd.indirect_dma_start(
                                out=cx[hp][:, g, :],
                                out_offset=None,
                                in_=cc_flat,
                                in_offset=bass.IndirectOffsetOnAxis(
                                    ap=cxidx_sb[:, g : g + 1], axis=0
                                ),
                            )
                            add_dep_helper(
                                gi.ins, cc_insts[hp].ins, sync=True,
                                reason="gather after A2A",
                            )
                        for dc in range(2):
                            for qs in range(4):
                                for g in range(4):
                                    nc.tensor.matmul(
                                        ops[dc * 4 + qs][:],
                                        cx[hp][:, g, qs * 128 : (qs + 1) * 128],
                                        wo_sb[:, hp * 4 + g, dc * QT : (dc + 1) * QT],
                                        start=(hp == 0 and g == 0),
                                        stop=(zero_bias and hp == 1 and g == 3),
                                    )
                    for dc in range(2):
                        for qs in range(4):
                            t = dc * 4 + qs
                            if not zero_bias:
                                nc.tensor.matmul(
                                    ops[t][:],
                                    ones_sb[:],
                                    bo_sb[:, dc * QT : (dc + 1) * QT],
                                    start=False,
                                    stop=True,
                                )
                            osb = osp.tile([128, QT], BF16, tag="osb")
                            nc.scalar.copy(osb[:], ops[t][:])
                            nc.gpsimd.dma_start(
                                out=out[
                                    qs * 128 : (qs + 1) * 128,
                                    dc * QT : (dc + 1) * QT,
                                ],
                                in_=osb[:],
                            )

    if not nc.is_finalized():
        nc.finalize()
    return nc


def _mask_blocks(mask):
    """Derive block structure + deduped additive mask tiles from the mask.

    Returns (score_blocks, mask_adds, ctx_blocks, n_masks), tiles:
      score_blocks[qc]: kc list where any of the [512q x 128k] tile is allowed
      mask_adds[qc]: (kc, qsub, tile_idx) for partially-masked 128x128 blocks
      ctx_blocks[qi]: kc list where any of the [128q x 128k] block is allowed
    """
    m = np.asarray(mask).reshape(S, S) != 0  # [q, k], True = allowed
    assert m.any(axis=1).all(), "rows with no allowed keys are unsupported"
    score_blocks = []
    mask_adds = []
    tiles = []
    tile_ids: dict = {}
    for qc in range(NQC):
        blk = []
        madd = []
        for kc in range(NKC):
            sub = m[qc * QT : (qc + 1) * QT, kc * KT : (kc + 1) * KT]
            if not sub.any():
                continue
            blk.append(kc)
            for qsub in range(QT // QS):
                ss = sub[qsub * QS : (qsub + 1) * QS]
                if ss.all() or not ss.any():
                    continue
                t = np.where(ss.T, 1.0, 0.0).astype(np.float32)
                key = t.tobytes()
                if key not in tile_ids:
                    tile_ids[key] = len(tiles)
                    tiles.append(t)
                madd.append((kc, qsub, tile_ids[key]))
        score_blocks.append(blk)
        mask_adds.append(madd)
    ctx_blocks = []
    for qi in range(NQI):
        cb = [
            kc
            for kc in range(NKC)
            if m[qi * QS : (qi + 1) * QS, kc * KT : (kc + 1) * KT].any()
        ]
        ctx_blocks.append(cb)
    return (score_blocks, mask_adds, ctx_blocks, max(len(tiles), 1)), tiles


def _prep_inputs(q, k, v, wq, bq, wk, bk, wv, bv, wo, bo, tiles):
    n_masks = max(len(tiles), 1)
    mka = (
        np.stack(tiles) if tiles else np.zeros((1, KT, QS), np.float32)
    ).astype(NPBF16)
    ident = np.eye(128, dtype=NPBF16)
    onesr = np.ones((1, 128), NPBF16)
    xt = {
        b: {
            "xq": np.ascontiguousarray(np.asarray(q[b]).T).astype(NPBF16),
            "xk": np.ascontiguousarray(np.asarray(k[b]).T).astype(NPBF16),
            "xv": np.ascontiguousarray(np.asarray(v[b]).T).astype(NPBF16),
        }
        for b in range(B)
    }
    # wo rows permuted so chunk jc = hp*4+g matches the cx gather layout:
    # row jc*128 + p  <->  global j = (4g + 2hp + p//64)*64 + p%64
    woT = np.ascontiguousarray(np.asarray(wo).T)  # [d_in=j, d_out]
    wo_prep = np.empty_like(woT)
    for hp in range(2):
        for g in range(4):
            for mm in range(2):
                head = 4 * g + 2 * hp + mm
                dst = (hp * 4 + g) * 128 + mm * 64
                wo_prep[dst : dst + 64] = woT[head * 64 : head * 64 + 64]
    wo_prep = wo_prep.astype(NPBF16)
    in_maps = []
    for c in range(NCORES):
        b, g = c // 4, c % 4
        js = slice(g * JPC, (g + 1) * JPC)
        # cx chunk gi=g2 comes from cc_all rows (4b+g2)*128 + p
        cxidx = np.empty((128, 4), np.int32)
        for p in range(128):
            for g2 in range(4):
                cxidx[p, g2] = (4 * b + g2) * 128 + p
        in_maps.append(
            {
                **xt[b],
                "wq": np.ascontiguousarray(np.asarray(wq)[js].T).astype(NPBF16),
                "wk": np.ascontiguousarray(np.asarray(wk)[js].T).astype(NPBF16),
                "wv": np.ascontiguousarray(np.asarray(wv)[js].T).astype(NPBF16),
                "wo": wo_prep,
                "cxidx": cxidx,
                "maskadd": mka,
                "ident": ident,
                "onesrow": onesr,
                "onescol": np.ones((128, 64), NPBF16),
                "bq": np.asarray(bq, np.float32)[js].reshape(2, 128).T.copy(),
                "bk": np.asarray(bk, np.float32)[js].reshape(2, 128).T.copy(),
                "bv": np.asarray(bv, np.float32)[js].reshape(1, JPC).astype(NPBF16),
                "bo": np.asarray(bo, np.float32).reshape(1, D).astype(NPBF16),
            }
        )
    return in_maps, n_masks


def kernel(q, k, v, mask, wq, bq, wk, bk, wv, bv, wo, bo, _reps=1):
    q = np.asarray(q, np.float32)
    k = np.asarray(k, np.float32)
    v = np.asarray(v, np.float32)
    mask_info, tiles = _mask_blocks(mask)
    in_maps, n_masks = _prep_inputs(q, k, v, wq, bq, wk, bk, wv, bv, wo, bo, tiles)
    zero_bias = not (np.any(np.asarray(bv)) or np.any(np.asarray(bo)))
    key = (str(mask_info), _reps, zero_bias)
    if key not in _CACHE:
        _CACHE[key] = _build_program(mask_info, _reps, zero_bias)
    nc = _CACHE[key]
    res = run_bass_kernel_spmd(nc, in_maps, list(range(NCORES)))
    out = np.empty((B, S, D), np.float32)
    for c in range(NCORES):
        b, qq = c // 4, c % 4
        out[b, qq * 512 : (qq + 1) * 512, :] = np.asarray(
            res.results[c]["o"], dtype=np.float32
        )
    return out
